# revision 4
# baseline (speedup 1.0000x reference)
"""Trainium2 Bass kernel for nn_ExactDivergenceModel (retrieval_knn).

Math (per batch b):
  XX[i,j] = ||X[i]-X[j]||, YX[i,j] = ||X[i]-Y[j]||
  out[b]  = (1/N) sum_i ( log min_{j!=i} XX[i,j] - log min_j YX[i,j] )
which only needs per-row minima of the squared-distance matrices:
  d2_XX[i,j] = x2[j] - 2<X_i,X_j>  (+ x2[i] added on host)
  d2_YX[i,j] = y2[j] - 2<X_i,Y_j>  (+ x2[i] added on host)

Device strategy (1 batch per NeuronCore, 8 cores):
  - Augmented matmul, K = D+2 = 66: lhsT = [-2*X^T; 1; 1], rhs = [R^T; r2_hi;
    r2_lo] so PSUM directly holds r2[j] - 2<X_i, R_j>. fp32r matmuls.
  - Diagonal of XX masked by accumulating BIG*I via an extra matmul
    (lhsT = rhs = sqrt(BIG)*I_128, start=False) - PE-only, no vector cost.
  - PSUM is split into two [128, 2048] slots used round-robin: the PE fills
    slot t%2 while the DVE min-reduces slot (t-1)%2, so PE and DVE overlap
    instead of strictly alternating (a single [128,4096] slot serializes
    them and measures ~2.2x slower).
  - Row minima via VectorE tensor_reduce(min); host combines the two
    half-row minima, adds x2[i], applies eps clamp + log + mean in float64.
  - Raw-bacc build: semaphores ride on the compute instructions (then_inc)
    with standalone waits only; no Tile-scheduler overhead.
  - I/O is latency-bound on this axon tunnel, so both distance matrices
    ship as ONE input tensor RXY [K, 2N] (single upload per call; the
    constant EYE upload is cached across calls) and both minima tensors
    return as ONE output MOUT [P, 128] (single download).

Execution: a jitted shard_map(bass_exec) callable is cached at module level
so repeat kernel() calls skip retracing/relowering (the lowering embeds the
NEFF and costs ~0.5 s per call otherwise). Fallback path goes through
run_bass_kernel_spmd, then per-core retry.
"""
import sys, time
sys.path.insert(0, '/opt/trn_rl_repo')

import numpy as np
import ml_dtypes

import concourse.bass as bass
from concourse import bacc, mybir
from concourse.bass_utils import run_bass_kernel_spmd

B, N, D = 8, 4096, 64
P = 128                 # partitions / i-block size
NB = N // P             # 32 i-blocks
K = D + 2               # contraction with the two norm rows
HALF = 2048             # psum slot width (two slots)
MMW = 512               # matmul free-dim width (one PSUM bank)
N_HALF = N // HALF      # 2 reduce pieces per (block, matrix)
EPS = 1e-12
SQRT_BIG = 32768.0      # BIG = 2^30 on the XX diagonal
f32 = mybir.dt.float32

_cache = {}


def _build(repeat=1):
    """Raw-bacc program. Per (block bi, matrix m in {XX, YX}) the [P, N]
    distance-row tile is computed as two [P, HALF] psum pieces; piece t goes
    to psum slot t%2, is min-reduced by DVE into mins[:, m*64 + 2*bi + h],
    and the PE may refill a slot only after the reduce of the piece two
    steps back (wait dve_sem >= t-1), overlapping PE and DVE."""
    n_ch = HALF // MMW              # 4 matmuls per piece
    mmdt = mybir.dt.float32r

    nc = bacc.Bacc(None, target_bir_lowering=False)
    RXY_d = nc.dram_tensor("RXY", [K, 2 * N], f32, kind="ExternalInput")
    EYE_d = nc.dram_tensor("EYE", [P, P], f32, kind="ExternalInput")
    MOUT_d = nc.dram_tensor("MOUT", [P, 2 * NB * N_HALF], f32,
                            kind="ExternalOutput")

    n_tiles_total = 2 * NB * N_HALF * repeat

    from contextlib import ExitStack
    with ExitStack() as ctx:
        RXYf = ctx.enter_context(nc.sbuf_tensor([K, 2 * N], f32))
        Lf = ctx.enter_context(nc.sbuf_tensor([K, N], f32))
        EYEf = ctx.enter_context(nc.sbuf_tensor([P, P], f32))
        RXYr = ctx.enter_context(nc.sbuf_tensor([K, 2 * N], mmdt))
        Lr = ctx.enter_context(nc.sbuf_tensor([K, N], mmdt))
        # EYEr in mmdt (f32r), NOT bf16: mixing dtypes inside the PE stream
        # forces a pipeline flush at each of the 64 weight-dtype switches
        # per repeat (~25-35 us measured). 32768 is exact in f32r.
        EYEr = ctx.enter_context(nc.sbuf_tensor([P, P], mmdt))
        mins = ctx.enter_context(nc.sbuf_tensor([P, 2 * NB * N_HALF], f32))
        psum = ctx.enter_context(nc.psum_tensor([P, N], f32))
        dma_sem = ctx.enter_context(nc.semaphore())
        conv_sem = ctx.enter_context(nc.semaphore())
        pe_sem = ctx.enter_context(nc.semaphore())
        dve_sem = ctx.enter_context(nc.semaphore())
        block = ctx.enter_context(nc.Block())

        @block.sync
        def _(sync):
            sync.dma_start(out=RXYf[:], in_=RXY_d[:]).then_inc(dma_sem, 16)
            sync.dma_start(out=EYEf[:], in_=EYE_d[:]).then_inc(dma_sem, 16)
            sync.wait_ge(dve_sem, n_tiles_total)
            sync.dma_start(out=MOUT_d[:], in_=mins[:]).then_inc(dma_sem, 16)

        @block.vector
        def _(vector):
            vector.wait_ge(dma_sem, 32)
            nc.vector.memset(Lf[D:D + 2, :], 1.0)
            nc.vector.tensor_scalar_mul(Lf[0:D, :], RXYf[0:D, 0:N], -2.0)
            nc.vector.tensor_copy(RXYr[:], RXYf[:])
            nc.vector.tensor_copy(Lr[:], Lf[:])
            nc.vector.tensor_copy(EYEr[:], EYEf[:]).then_inc(conv_sem, 1)
            t = 0
            for _r in range(repeat):
                for bi in range(NB):
                    for m in range(2):
                        for h in range(N_HALF):
                            slot = t % 2
                            vector.wait_ge(pe_sem, t + 1)
                            col = m * NB * N_HALF + bi * N_HALF + h
                            nc.vector.tensor_reduce(
                                out=mins[:, col:col + 1],
                                in_=psum[:, slot * HALF:(slot + 1) * HALF],
                                axis=mybir.AxisListType.X,
                                op=mybir.AluOpType.min).then_inc(dve_sem, 1)
                            t += 1

        @block.tensor
        def _(tensor):
            tensor.wait_ge(conv_sem, 1)
            t = 0
            for _r in range(repeat):
                for bi in range(NB):
                    lhs = Lr[:, bi * P:(bi + 1) * P]
                    for m in range(2):          # 0: XX (cols 0:N), 1: YX
                        is_xx = m == 0
                        for h in range(N_HALF):
                            slot = t % 2
                            if t >= 2:
                                tensor.wait_ge(dve_sem, t - 1)
                            mm = None
                            for c in range(n_ch):
                                col0 = h * HALF + c * MMW
                                diag_here = (is_xx
                                             and col0 <= bi * P < col0 + MMW)
                                mm = nc.tensor.matmul(
                                    psum[:, slot * HALF + c * MMW:
                                         slot * HALF + (c + 1) * MMW],
                                    lhs,
                                    RXYr[:, m * N + col0:m * N + col0 + MMW],
                                    start=True, stop=not diag_here)
                                if diag_here:
                                    off = slot * HALF + (bi * P - h * HALF)
                                    mm = nc.tensor.matmul(
                                        psum[:, off:off + P], EYEr[:], EYEr[:],
                                        start=False, stop=True,
                                        skip_group_check=True)
                            mm.then_inc(pe_sem, 1)
                            t += 1

    nc.finalize()
    return nc


def _get_nc(repeat=1):
    key = ("raw2", repeat)
    if key not in _cache:
        _cache[key] = _build(repeat)
    return _cache[key]


def _eye_np():
    return (np.eye(P) * SQRT_BIG).astype(np.float32)


def _prep_maps(X, Y):
    X = np.asarray(X, dtype=np.float32)
    Y = np.asarray(Y, dtype=np.float32)
    eye = _eye_np()
    in_maps, x2_all = [], []
    for b in range(B):
        Xb = X[b].astype(np.float64)
        Yb = Y[b].astype(np.float64)
        x2 = (Xb * Xb).sum(1)
        y2 = (Yb * Yb).sum(1)
        # hi part must be exactly representable in bf16 (the f32r matmul's
        # coarsest pass); the residual rides in the second augmented row.
        x2h = x2.astype(np.float32).astype(ml_dtypes.bfloat16).astype(np.float64)
        y2h = y2.astype(np.float32).astype(ml_dtypes.bfloat16).astype(np.float64)
        RXY = np.empty((K, 2 * N), np.float32)
        RXY[0:D, 0:N] = Xb.T
        RXY[D, 0:N] = x2h
        RXY[D + 1, 0:N] = x2 - x2h
        RXY[0:D, N:] = Yb.T
        RXY[D, N:] = y2h
        RXY[D + 1, N:] = y2 - y2h
        in_maps.append({"RXY": RXY, "EYE": eye})
        x2_all.append(x2)
    return in_maps, x2_all


def _postprocess(results, x2_all):
    out = np.zeros(B, dtype=np.float64)
    for b in range(B):
        mout = results[b]["MOUT"].astype(np.float64)  # [P, 2*NB*N_HALF]
        mx, my = mout[:, :NB * N_HALF], mout[:, NB * N_HALF:]
        d2x = mx.reshape(P, NB, N_HALF).min(2).T.reshape(-1) + x2_all[b]
        d2y = my.reshape(P, NB, N_HALF).min(2).T.reshape(-1) + x2_all[b]
        d2x = np.maximum(d2x, EPS)
        d2y = np.maximum(d2y, EPS)
        out[b] = 0.5 * np.mean(np.log(d2x) - np.log(d2y))
    return out.astype(np.float32)


# ---------------------------------------------------------------------------
# Cached jitted runner: build the shard_map(bass_exec) callable once; per
# call only the changed inputs are device_put (EYE is constant and cached).

def _make_runner(nc, n_cores=B):
    import jax
    from jax.sharding import Mesh, PartitionSpec, NamedSharding
    from jax.experimental.shard_map import shard_map
    from concourse.bass2jax import (
        _bass_exec_p, install_neuronx_cc_hook, partition_id_tensor)

    install_neuronx_cc_hook()
    partition_name = (nc.partition_id_tensor.name
                      if nc.partition_id_tensor else None)
    in_names, out_names, out_avals, zero_outs = [], [], [], []
    for alloc in nc.m.functions[0].allocations:
        if not isinstance(alloc, mybir.MemoryLocationSet):
            continue
        name = alloc.memorylocations[0].name
        if alloc.kind == "ExternalInput":
            if name != partition_name:
                in_names.append(name)
        elif alloc.kind == "ExternalOutput":
            out_names.append(name)
            shape = tuple(alloc.tensor_shape)
            dtype = mybir.dt.np(alloc.dtype)
            out_avals.append(jax.core.ShapedArray(shape, dtype))
            zero_outs.append(np.zeros(shape, dtype))
    n_params = len(in_names)
    in_names_all = list(in_names) + out_names
    if partition_name is not None:
        in_names_all.append(partition_name)

    def _body(*args):
        operands = list(args)
        if partition_name is not None:
            operands.append(partition_id_tensor())
        outs = _bass_exec_p.bind(
            *operands,
            out_avals=tuple(out_avals),
            in_names=tuple(in_names_all),
            out_names=tuple(out_names),
            lowering_input_output_aliases=(),
            sim_require_finite=True,
            sim_require_nnan=True,
            nc=nc,
        )
        return tuple(outs)

    devices = jax.devices()[:n_cores]
    mesh = Mesh(np.asarray(devices), ("core",))
    in_specs = (PartitionSpec("core"),) * (n_params + len(out_names))
    out_specs = (PartitionSpec("core"),) * len(out_names)
    fn = jax.jit(
        shard_map(_body, mesh=mesh, in_specs=in_specs, out_specs=out_specs,
                  check_rep=False),
        keep_unused=True,
    )
    sharding = NamedSharding(mesh, PartitionSpec("core"))
    dev_zeros = [
        jax.device_put(
            np.zeros((n_cores * z.shape[0], *z.shape[1:]), z.dtype), sharding)
        for z in zero_outs
    ]
    return dict(fn=fn, in_names=in_names, out_names=out_names,
                out_avals=out_avals, sharding=sharding, dev_zeros=dev_zeros,
                n_cores=n_cores, const_dev={})


def _get_runner(repeat=1):
    key = ("runner", repeat)
    if key not in _cache:
        _cache[key] = _make_runner(_get_nc(repeat))
    return _cache[key]


def _run_cached(in_maps):
    import jax
    st = _get_runner(1)
    n_cores = st["n_cores"]
    dev_in = []
    for name in st["in_names"]:
        if name == "EYE":
            if name not in st["const_dev"]:
                a = np.concatenate([np.asarray(in_maps[c][name])
                                    for c in range(n_cores)], axis=0)
                st["const_dev"][name] = jax.device_put(a, st["sharding"])
            dev_in.append(st["const_dev"][name])
        else:
            a = np.concatenate([np.asarray(in_maps[c][name])
                                for c in range(n_cores)], axis=0)
            dev_in.append(jax.device_put(a, st["sharding"]))
    out = st["fn"](*dev_in, *st["dev_zeros"])
    jax.block_until_ready(out)
    return [
        {name: np.asarray(out[i]).reshape(n_cores, *st["out_avals"][i].shape)[c]
         for i, name in enumerate(st["out_names"])}
        for c in range(n_cores)
    ]


def _run_with_retry(nc, in_maps):
    for attempt in range(2):
        try:
            return run_bass_kernel_spmd(nc, in_maps,
                                        core_ids=list(range(B))).results
        except Exception:
            time.sleep(3)
    # last resort: one batch at a time, skipping wedged cores
    results = [None] * B
    for b in range(B):
        for c in range(8):
            core = (b + c) % 8
            try:
                results[b] = run_bass_kernel_spmd(
                    nc, [in_maps[b]], core_ids=[core]).results[0]
                break
            except Exception:
                continue
        if results[b] is None:
            raise RuntimeError("all cores failed")
    return results


def kernel(X, Y):
    in_maps, x2_all = _prep_maps(X, Y)
    try:
        results = _run_cached(in_maps)
    except Exception:
        results = _run_with_retry(_get_nc(1), in_maps)
    return _postprocess(results, x2_all)


# Pre-build the program at import time so the first kernel() call doesn't pay
# Bass graph construction; guarded so import can never fail.
try:
    _get_nc(1)
except Exception:
    pass


if __name__ == "__main__":
    rng = np.random.default_rng(0)
    X = rng.standard_normal((B, N, D)).astype(np.float32)
    Y = rng.standard_normal((B, N, D)).astype(np.float32)
    print(kernel(X, Y))


# revision 9
# speedup vs baseline: 1.3087x; 1.3087x over previous
"""Trainium2 Bass kernel for nn_ExactDivergenceModel (retrieval_knn).

Math (per batch b):
  XX[i,j] = ||X[i]-X[j]||, YX[i,j] = ||X[i]-Y[j]||
  out[b]  = (1/N) sum_i ( log min_{j!=i} XX[i,j] - log min_j YX[i,j] )
which only needs per-row minima of the squared-distance matrices:
  d2_XX[i,j] = x2[j] - 2<X_i,X_j>  (+ x2[i] added on host)
  d2_YX[i,j] = y2[j] - 2<X_i,Y_j>  (+ x2[i] added on host)

Device strategy (1 batch per NeuronCore, 8 cores):
  - Augmented matmul, K = D+2 = 66: lhsT = [-2*X^T; 1; 1], rhs = [R^T; r2_hi;
    r2_lo] so PSUM directly holds r2[j] - 2<X_i, R_j>. fp32r matmuls.
  - Diagonal of XX masked by accumulating BIG*I via an extra matmul
    (lhsT = rhs = sqrt(BIG)*I_128, start=False) - PE-only, no vector cost.
  - PSUM is split into two [128, 2048] slots used round-robin: the PE fills
    slot t%2 while the DVE min-reduces slot (t-1)%2, so PE and DVE overlap
    instead of strictly alternating (a single [128,4096] slot serializes
    them and measures ~2.2x slower).
  - Row minima via VectorE tensor_reduce(min); host combines the two
    half-row minima, adds x2[i], applies eps clamp + log + mean in float64.
  - Raw-bacc build: semaphores ride on the compute instructions (then_inc)
    with standalone waits only; no Tile-scheduler overhead.
  - I/O is latency-bound on this axon tunnel, so both distance matrices
    ship as ONE input tensor RXY [K, 2N] (single upload per call; the
    constant EYE upload is cached across calls) and both minima tensors
    return as ONE output MOUT [P, 128] (single download).

Execution: a jitted shard_map(bass_exec) callable is cached at module level
so repeat kernel() calls skip retracing/relowering (the lowering embeds the
NEFF and costs ~0.5 s per call otherwise). Fallback path goes through
run_bass_kernel_spmd, then per-core retry.
"""
import sys, time
sys.path.insert(0, '/opt/trn_rl_repo')

import numpy as np
import ml_dtypes

import concourse.bass as bass
from concourse import bacc, mybir
from concourse.bass_utils import run_bass_kernel_spmd

B, N, D = 8, 4096, 64
P = 128                 # partitions / i-block size
NB = N // P             # 32 i-blocks
K = D + 2               # contraction with the two norm rows
HALF = 2048             # psum slot width (two slots)
MMW = 512               # matmul free-dim width (one PSUM bank)
N_HALF = N // HALF      # 2 reduce pieces per (block, matrix)
EPS = 1e-12
SQRT_BIG = 32768.0      # BIG = 2^30 on the XX diagonal
f32 = mybir.dt.float32

_cache = {}


def _build(repeat=1):
    """Raw-bacc program. Per (block bi, matrix m in {XX, YX}) the [P, N]
    distance-row tile is computed as two [P, HALF] psum pieces; piece t goes
    to psum slot t%2, is min-reduced by DVE into mins[:, m*64 + 2*bi + h],
    and the PE may refill a slot only after the reduce of the piece two
    steps back (wait dve_sem >= t-1), overlapping PE and DVE."""
    n_ch = HALF // MMW              # 4 matmuls per piece
    mmdt = mybir.dt.float32r

    nc = bacc.Bacc(None, target_bir_lowering=False)
    RXY_d = nc.dram_tensor("RXY", [K, 2 * N], f32, kind="ExternalInput")
    EYE_d = nc.dram_tensor("EYE", [P, P], f32, kind="ExternalInput")
    MOUT_d = nc.dram_tensor("MOUT", [P, 2 * NB * N_HALF], f32,
                            kind="ExternalOutput")

    n_tiles_total = 2 * NB * N_HALF * repeat

    from contextlib import ExitStack
    with ExitStack() as ctx:
        RXYf = ctx.enter_context(nc.sbuf_tensor([K, 2 * N], f32))
        Lf = ctx.enter_context(nc.sbuf_tensor([K, N], f32))
        EYEf = ctx.enter_context(nc.sbuf_tensor([P, P], f32))
        RXYr = ctx.enter_context(nc.sbuf_tensor([K, 2 * N], mmdt))
        Lr = ctx.enter_context(nc.sbuf_tensor([K, N], mmdt))
        # EYEr in mmdt (f32r), NOT bf16: mixing dtypes inside the PE stream
        # forces a pipeline flush at each of the 64 weight-dtype switches
        # per repeat (~25-35 us measured). 32768 is exact in f32r.
        EYEr = ctx.enter_context(nc.sbuf_tensor([P, P], mmdt))
        EYEnf = ctx.enter_context(nc.sbuf_tensor([P, P], f32))
        EYEnr = ctx.enter_context(nc.sbuf_tensor([P, P], mmdt))
        mins = ctx.enter_context(nc.sbuf_tensor([P, 2 * NB * N_HALF], f32))
        psum = ctx.enter_context(nc.psum_tensor([P, N], f32))
        dma_sem = ctx.enter_context(nc.semaphore())
        conv_sem = ctx.enter_context(nc.semaphore())
        pe_sem = ctx.enter_context(nc.semaphore())
        dve_sem = ctx.enter_context(nc.semaphore())
        block = ctx.enter_context(nc.Block())

        @block.sync
        def _(sync):
            sync.dma_start(out=RXYf[:], in_=RXY_d[:]).then_inc(dma_sem, 16)
            sync.dma_start(out=EYEf[:], in_=EYE_d[:]).then_inc(dma_sem, 16)
            sync.wait_ge(dve_sem, n_tiles_total)
            sync.dma_start(out=MOUT_d[:], in_=mins[:]).then_inc(dma_sem, 16)

        @block.vector
        def _(vector):
            vector.wait_ge(dma_sem, 32)
            # NEGATED distances: psum holds 2<x_i,r_j> - r2[j] = -d2_raw, so
            # the row reduction is a MAX, served by pool_max (~300 ns less
            # per-instruction overhead than tensor_reduce(min) on the DVE,
            # which is the saturated engine).
            nc.vector.memset(Lf[D:D + 2, :], -1.0)
            nc.vector.tensor_scalar_mul(Lf[0:D, :], RXYf[0:D, 0:N], 2.0)
            nc.vector.tensor_copy(RXYr[:], RXYf[:])
            nc.vector.tensor_copy(Lr[:], Lf[:])
            nc.vector.tensor_scalar_mul(EYEnf[:], EYEf[:], -1.0)
            nc.vector.tensor_copy(EYEnr[:], EYEnf[:])
            nc.vector.tensor_copy(EYEr[:], EYEf[:]).then_inc(conv_sem, 1)
            t = 0
            for _r in range(repeat):
                for bi in range(NB):
                    for m in range(2):
                        for h in range(N_HALF):
                            slot = t % 2
                            vector.wait_ge(pe_sem, t + 1)
                            col = m * NB * N_HALF + bi * N_HALF + h
                            nc.vector.pool_max(
                                out=mins[:, col:col + 1],
                                in_=psum[:, slot * HALF:(slot + 1) * HALF],
                            ).then_inc(dve_sem, 1)
                            t += 1

        @block.tensor
        def _(tensor):
            tensor.wait_ge(conv_sem, 1)
            t = 0
            for _r in range(repeat):
                for bi in range(NB):
                    lhs = Lr[:, bi * P:(bi + 1) * P]
                    for m in range(2):          # 0: XX (cols 0:N), 1: YX
                        is_xx = m == 0
                        for h in range(N_HALF):
                            slot = t % 2
                            if t >= 2:
                                tensor.wait_ge(dve_sem, t - 1)
                            mm = None
                            for c in range(n_ch):
                                col0 = h * HALF + c * MMW
                                diag_here = (is_xx
                                             and col0 <= bi * P < col0 + MMW)
                                mm = nc.tensor.matmul(
                                    psum[:, slot * HALF + c * MMW:
                                         slot * HALF + (c + 1) * MMW],
                                    lhs,
                                    RXYr[:, m * N + col0:m * N + col0 + MMW],
                                    start=True, stop=not diag_here)
                                if diag_here:
                                    # accumulate -BIG on the diagonal so the
                                    # MAX of -d2 never selects the self-pair
                                    off = slot * HALF + (bi * P - h * HALF)
                                    mm = nc.tensor.matmul(
                                        psum[:, off:off + P], EYEr[:], EYEnr[:],
                                        start=False, stop=True,
                                        skip_group_check=True)
                            mm.then_inc(pe_sem, 1)
                            t += 1

    nc.finalize()
    return nc


def _get_nc(repeat=1):
    key = ("raw2", repeat)
    if key not in _cache:
        _cache[key] = _build(repeat)
    return _cache[key]


def _eye_np():
    return (np.eye(P) * SQRT_BIG).astype(np.float32)


def _prep_maps(X, Y):
    X = np.asarray(X, dtype=np.float32)
    Y = np.asarray(Y, dtype=np.float32)
    eye = _eye_np()
    in_maps, x2_all = [], []
    for b in range(B):
        Xb = X[b].astype(np.float64)
        Yb = Y[b].astype(np.float64)
        x2 = (Xb * Xb).sum(1)
        y2 = (Yb * Yb).sum(1)
        # hi part must be exactly representable in bf16 (the f32r matmul's
        # coarsest pass); the residual rides in the second augmented row.
        x2h = x2.astype(np.float32).astype(ml_dtypes.bfloat16).astype(np.float64)
        y2h = y2.astype(np.float32).astype(ml_dtypes.bfloat16).astype(np.float64)
        RXY = np.empty((K, 2 * N), np.float32)
        RXY[0:D, 0:N] = Xb.T
        RXY[D, 0:N] = x2h
        RXY[D + 1, 0:N] = x2 - x2h
        RXY[0:D, N:] = Yb.T
        RXY[D, N:] = y2h
        RXY[D + 1, N:] = y2 - y2h
        in_maps.append({"RXY": RXY, "EYE": eye})
        x2_all.append(x2)
    return in_maps, x2_all


def _postprocess(results, x2_all):
    out = np.zeros(B, dtype=np.float64)
    for b in range(B):
        mout = results[b]["MOUT"].astype(np.float64)  # [P, 2*NB*N_HALF]
        mx, my = mout[:, :NB * N_HALF], mout[:, NB * N_HALF:]
        # device holds max(-d2_raw) partials: combine with max, then negate
        d2x = x2_all[b] - mx.reshape(P, NB, N_HALF).max(2).T.reshape(-1)
        d2y = x2_all[b] - my.reshape(P, NB, N_HALF).max(2).T.reshape(-1)
        d2x = np.maximum(d2x, EPS)
        d2y = np.maximum(d2y, EPS)
        out[b] = 0.5 * np.mean(np.log(d2x) - np.log(d2y))
    return out.astype(np.float32)


# ---------------------------------------------------------------------------
# Cached jitted runner: build the shard_map(bass_exec) callable once; per
# call only the changed inputs are device_put (EYE is constant and cached).

def _make_runner(nc, n_cores=B):
    import jax
    from jax.sharding import Mesh, PartitionSpec, NamedSharding
    from jax.experimental.shard_map import shard_map
    from concourse.bass2jax import (
        _bass_exec_p, install_neuronx_cc_hook, partition_id_tensor)

    install_neuronx_cc_hook()
    partition_name = (nc.partition_id_tensor.name
                      if nc.partition_id_tensor else None)
    in_names, out_names, out_avals, zero_outs = [], [], [], []
    for alloc in nc.m.functions[0].allocations:
        if not isinstance(alloc, mybir.MemoryLocationSet):
            continue
        name = alloc.memorylocations[0].name
        if alloc.kind == "ExternalInput":
            if name != partition_name:
                in_names.append(name)
        elif alloc.kind == "ExternalOutput":
            out_names.append(name)
            shape = tuple(alloc.tensor_shape)
            dtype = mybir.dt.np(alloc.dtype)
            out_avals.append(jax.core.ShapedArray(shape, dtype))
            zero_outs.append(np.zeros(shape, dtype))
    n_params = len(in_names)
    in_names_all = list(in_names) + out_names
    if partition_name is not None:
        in_names_all.append(partition_name)

    def _body(*args):
        operands = list(args)
        if partition_name is not None:
            operands.append(partition_id_tensor())
        outs = _bass_exec_p.bind(
            *operands,
            out_avals=tuple(out_avals),
            in_names=tuple(in_names_all),
            out_names=tuple(out_names),
            lowering_input_output_aliases=(),
            sim_require_finite=True,
            sim_require_nnan=True,
            nc=nc,
        )
        return tuple(outs)

    devices = jax.devices()[:n_cores]
    mesh = Mesh(np.asarray(devices), ("core",))
    in_specs = (PartitionSpec("core"),) * (n_params + len(out_names))
    out_specs = (PartitionSpec("core"),) * len(out_names)
    fn = jax.jit(
        shard_map(_body, mesh=mesh, in_specs=in_specs, out_specs=out_specs,
                  check_rep=False),
        keep_unused=True,
    )
    sharding = NamedSharding(mesh, PartitionSpec("core"))
    dev_zeros = [
        jax.device_put(
            np.zeros((n_cores * z.shape[0], *z.shape[1:]), z.dtype), sharding)
        for z in zero_outs
    ]
    return dict(fn=fn, in_names=in_names, out_names=out_names,
                out_avals=out_avals, sharding=sharding, dev_zeros=dev_zeros,
                n_cores=n_cores, const_dev={})


def _get_runner(repeat=1):
    key = ("runner", repeat)
    if key not in _cache:
        _cache[key] = _make_runner(_get_nc(repeat))
    return _cache[key]


def _run_cached(in_maps):
    import jax
    st = _get_runner(1)
    n_cores = st["n_cores"]
    dev_in = []
    for name in st["in_names"]:
        if name == "EYE":
            if name not in st["const_dev"]:
                a = np.concatenate([np.asarray(in_maps[c][name])
                                    for c in range(n_cores)], axis=0)
                st["const_dev"][name] = jax.device_put(a, st["sharding"])
            dev_in.append(st["const_dev"][name])
        else:
            a = np.concatenate([np.asarray(in_maps[c][name])
                                for c in range(n_cores)], axis=0)
            dev_in.append(jax.device_put(a, st["sharding"]))
    out = st["fn"](*dev_in, *st["dev_zeros"])
    jax.block_until_ready(out)
    return [
        {name: np.asarray(out[i]).reshape(n_cores, *st["out_avals"][i].shape)[c]
         for i, name in enumerate(st["out_names"])}
        for c in range(n_cores)
    ]


def _run_with_retry(nc, in_maps):
    for attempt in range(2):
        try:
            return run_bass_kernel_spmd(nc, in_maps,
                                        core_ids=list(range(B))).results
        except Exception:
            time.sleep(3)
    # last resort: one batch at a time, skipping wedged cores
    results = [None] * B
    for b in range(B):
        for c in range(8):
            core = (b + c) % 8
            try:
                results[b] = run_bass_kernel_spmd(
                    nc, [in_maps[b]], core_ids=[core]).results[0]
                break
            except Exception:
                continue
        if results[b] is None:
            raise RuntimeError("all cores failed")
    return results


def kernel(X, Y):
    in_maps, x2_all = _prep_maps(X, Y)
    try:
        results = _run_cached(in_maps)
    except Exception:
        results = _run_with_retry(_get_nc(1), in_maps)
    return _postprocess(results, x2_all)


# Pre-build the program at import time so the first kernel() call doesn't pay
# Bass graph construction; guarded so import can never fail.
try:
    _get_nc(1)
except Exception:
    pass


if __name__ == "__main__":
    rng = np.random.default_rng(0)
    X = rng.standard_normal((B, N, D)).astype(np.float32)
    Y = rng.standard_normal((B, N, D)).astype(np.float32)
    print(kernel(X, Y))
